# revision 27
# baseline (speedup 1.0000x reference)
"""Trainium2 Bass kernel for nn_CapsuleLayer (capsule conv + 3-iter routing).

Reference (per batch image, C=128, H=W=32, K=3, pad=1):
  priors[h,w,t,nc] = sum_c x_pad[c,h+i,w+j] * W[t,c,nc] + b[t,nc]
  o = mean_t priors
  3x: d2 = sum_cch (o - p_t)^2 ; cw = rsqrt(d2 + 1e-4)
      cw = cw / sum_t cw ; o = sum_t cw_t p_t
  out[nc,h,w] = o

Sharding: data-parallel over batch; 8 cores, one image each; weight/bias
replicated; no collectives.

Implementation notes (v2):
- bf16 everywhere on-chip (fp32 PSUM accumulation in matmuls); rel err vs
  fp32 reference ~5e-3, within the 2e-2 gate.
- priors layout [128pos, tap9, cch16, cap32] so every big DVE op runs in
  2x_1P mode (innermost step-1 cap runs, broadcasts on outer/mid dims).
- ||o - p||^2 = ||p||^2 - <2o, p> + ||o||^2: the only per-iteration
  full-size passes are the product o2*p, its cch-reduction, the weighted
  product p*alpha and its tap-reduction. Reductions are halving adds
  (2x mode) instead of 1x tensor_reduce.
- mean_t priors comes from 9 extra accumulating matmuls on the idle PE.
- rsqrt via exp(-0.5*ln(x)) on the Scalar engine (sanctioned rsqrt path
  is DVE-reciprocal-heavy; cw is scale-invariant after normalization).
- ||o||^2 = sum_t alpha_t <o, p_t> reuses the s-values (no extra pass).
"""

import numpy as np

C = 128
H = W = 32
B = 8
KK = 9
NCAPS = 32
CCH = 16
NC = NCAPS * CCH  # 512
NIT = 3
NPOS = H * W
CHUNK = 128
NCHUNK = NPOS // CHUNK
PADW = 34

_cache = {}


def _build(with_bias: bool):
    import concourse.bass as bass
    import concourse.tile as tile
    from concourse import bacc, mybir
    from concourse.masks import make_identity

    f32 = mybir.dt.float32
    bf16 = mybir.dt.bfloat16
    X = mybir.AxisListType.X
    ADD = mybir.AluOpType.add
    AF = mybir.ActivationFunctionType

    nc = bacc.Bacc(num_swdge_queues=4)
    ACC = mybir.AluOpType.add
    x_d = nc.dram_tensor("x", [C, H, W], f32, kind="ExternalInput")
    w_d = nc.dram_tensor("w", [KK, C, NC], f32, kind="ExternalInput")
    b_d = nc.dram_tensor("b", [KK, NC], f32, kind="ExternalInput")
    out_d = nc.dram_tensor("out", [NC, NPOS], f32, kind="ExternalOutput")

    with tile.TileContext(nc) as tc:
        with (
            tc.tile_pool(name="singles", bufs=1) as singles,
            tc.tile_pool(name="priors", bufs=3) as priors_pool,
            tc.tile_pool(name="big", bufs=2) as big_pool,
            tc.tile_pool(name="half", bufs=2) as half_pool,
            tc.tile_pool(name="o", bufs=2) as o_pool,
            tc.tile_pool(name="small", bufs=4) as small_pool,
            tc.tile_pool(name="pp", bufs=4, space="PSUM") as pp,
            tc.tile_pool(name="mp", bufs=2, space="PSUM") as mp,
            tc.tile_pool(name="tpp", bufs=2, space="PSUM") as tpp,
        ):
            # ---- stage inputs: pad, cast to bf16, im2col, permute W ----
            xpadf = singles.tile([C, PADW * PADW], f32)
            nc.gpsimd.memset(xpadf, 0.0)
            xpadf_v = xpadf[:].rearrange("p (h w) -> p h w", h=PADW)
            nc.sync.dma_start(out=xpadf_v[:, 1 : H + 1, 1 : W + 1], in_=x_d[:])
            xpad = singles.tile([C, PADW * PADW], bf16)
            nc.scalar.copy(out=xpad[:], in_=xpadf[:])
            xpad_v = xpad[:].rearrange("p (h w) -> p h w", h=PADW)

            xcol = []
            for t in range(KK):
                i, j = divmod(t, 3)
                xc = singles.tile([C, NPOS], bf16, tag=f"xcol{t}")
                nc.sync.dma_start(
                    out=xc[:].rearrange("p (h w) -> p h w", h=H),
                    in_=xpad_v[:, i : i + H, j : j + W],
                )
                xcol.append(xc)

            wraw = singles.tile([C, KK, NC], f32)
            nc.sync.dma_start(out=wraw[:], in_=w_d[:].transpose([1, 0, 2]))
            # wsb[c, t, cch, cap] (bf16) <- wraw[c, t, cap*16+cch]
            wsb = singles.tile([C, KK, CCH, NCAPS], bf16)
            nc.scalar.copy(
                out=wsb[:],
                in_=wraw[:].rearrange("p t (cap cch) -> p t cch cap", cch=CCH),
            )

            ident = singles.tile([128, 128], f32)
            make_identity(nc, ident[:])

            if with_bias:
                braw = singles.tile([1, KK, NC], f32)
                nc.sync.dma_start(out=braw[:], in_=b_d[:].unsqueeze(0))
                bsb = singles.tile([1, KK, CCH, NCAPS], bf16)
                nc.scalar.copy(
                    out=bsb[:],
                    in_=braw[:].rearrange("p t (cap cch) -> p t cch cap", cch=CCH),
                )
                ones = singles.tile([1, CHUNK], bf16)
                nc.gpsimd.memset(ones, 1.0)

            for ch in range(NCHUNK):
                # ---- priors + mean via PE ----
                priors = priors_pool.tile([128, KK, CCH, NCAPS], bf16)
                om = mp.tile([128, NC], f32)  # sum_t priors (fp32 psum)
                for t in range(KK):
                    ps = pp.tile([128, NC], f32)
                    lhsT = xcol[t][:, CHUNK * ch : CHUNK * (ch + 1)]
                    rhs = wsb[:, t].rearrange("p a b -> p (a b)")
                    if with_bias:
                        nc.tensor.matmul(ps[:], lhsT, rhs, start=True, stop=False)
                        brhs = bsb[:, t].rearrange("p a b -> p (a b)")
                        nc.tensor.matmul(ps[:], ones[:], brhs, start=False, stop=True)
                    else:
                        nc.tensor.matmul(ps[:], lhsT, rhs, start=True, stop=True)
                    nc.tensor.matmul(
                        om[:], lhsT, rhs, start=(t == 0), stop=(t == KK - 1)
                    )
                    if with_bias:
                        # mean accumulates bias once per tap as well
                        nc.tensor.matmul(
                            om[:], ones[:], brhs, start=False, stop=False,
                            skip_group_check=True,
                        )
                    nc.scalar.copy(
                        out=priors[:, t],
                        in_=ps[:].rearrange("p (a b) -> p a b", a=CCH),
                    )

                # o2 = 2*mean = (2/9) * sum_t priors   [128, (cch,cap)] bf16
                o2 = o_pool.tile([128, NC], bf16)
                nc.scalar.activation(
                    out=o2[:], in_=om[:], func=AF.Copy, scale=2.0 / KK
                )

                # ---- n[t,cap] = sum_cch p^2 (one-time per chunk) ----
                tprod = big_pool.tile([128, KK, CCH, NCAPS], bf16)
                nc.scalar.activation(out=tprod[:], in_=priors[:], func=AF.Square)
                # cch reduction: big steps on DMA engines (accum), tail on DVE
                nc.gpsimd.dma_start(
                    out=tprod[:, :, 0:8, :], in_=tprod[:, :, 8:16, :], accum_op=ACC
                )
                nc.gpsimd.dma_start(
                    out=tprod[:, :, 0:4, :], in_=tprod[:, :, 4:8, :], accum_op=ACC
                )
                h1 = half_pool.tile([128, KK, 2, NCAPS], bf16)
                nc.vector.tensor_add(h1[:], tprod[:, :, 0:2, :], tprod[:, :, 2:4, :])
                ntile = small_pool.tile([128, KK, NCAPS], bf16, tag="n")
                nc.vector.tensor_add(ntile[:], h1[:, :, 0], h1[:, :, 1])

                alpha = None  # bf16 [128, KK, NCAPS]; None => uniform 1/9
                for it in range(NIT):
                    last = it == NIT - 1
                    # s = <o2, p_t> per (tap, cap): product + cch halving
                    tprod = big_pool.tile([128, KK, CCH, NCAPS], bf16)
                    ob = (
                        o2[:]
                        .rearrange("p (a b) -> p a b", a=CCH)
                        .unsqueeze(1)
                        .broadcast_to((128, KK, CCH, NCAPS))
                    )
                    nc.vector.tensor_mul(tprod[:], priors[:], ob)
                    nc.gpsimd.dma_start(
                        out=tprod[:, :, 0:8, :],
                        in_=tprod[:, :, 8:16, :],
                        accum_op=ACC,
                    )
                    nc.gpsimd.dma_start(
                        out=tprod[:, :, 0:4, :],
                        in_=tprod[:, :, 4:8, :],
                        accum_op=ACC,
                    )
                    h1 = half_pool.tile([128, KK, 2, NCAPS], bf16)
                    nc.vector.tensor_add(
                        h1[:], tprod[:, :, 0:2, :], tprod[:, :, 2:4, :]
                    )
                    s = small_pool.tile([128, KK, NCAPS], bf16, tag="s")
                    nc.vector.tensor_add(s[:], h1[:, :, 0], h1[:, :, 1])

                    # e2 = sum_t alpha_t * s_t ; e = ||o||^2 (+eps folded)
                    e2 = small_pool.tile([128, NCAPS], f32, tag="e2")
                    if alpha is None:
                        nc.vector.tensor_reduce(
                            out=e2[:], in_=s[:].transpose([0, 2, 1]), axis=X, op=ADD
                        )
                        # e' = e2/(2*9) + eps
                        nc.vector.tensor_scalar(
                            e2[:], e2[:], 1.0 / (2 * KK), 1e-4,
                            op0=mybir.AluOpType.mult, op1=ADD,
                        )
                    else:
                        tm = small_pool.tile([128, KK, NCAPS], f32, tag="tm")
                        nc.vector.tensor_mul(tm[:], alpha[:], s[:])
                        nc.vector.tensor_reduce(
                            out=e2[:], in_=tm[:].transpose([0, 2, 1]), axis=X, op=ADD
                        )
                        # o2 = sum alpha2*p with alpha2 = 2*alpha_norm
                        # => e = ||o||^2 = (1/4) sum alpha2 <o2, p> = e2/4
                        nc.vector.tensor_scalar(
                            e2[:], e2[:], 0.25, 1e-4,
                            op0=mybir.AluOpType.mult, op1=ADD,
                        )

                    # dist = (n - s) + e'  (fp32)
                    dist = small_pool.tile([128, KK, NCAPS], f32, tag="dist")
                    nc.vector.tensor_sub(dist[:], ntile[:], s[:])
                    nc.vector.tensor_add(
                        dist[:],
                        dist[:],
                        e2[:].unsqueeze(1).broadcast_to((128, KK, NCAPS)),
                    )
                    # cwu = dist^-0.5 on ACT (abs_reciprocal_sqrt set also
                    # holds square+copy -> single table set, no thrash)
                    cwu = small_pool.tile([128, KK, NCAPS], bf16, tag="cwu")
                    nc.scalar.activation(
                        out=cwu[:], in_=dist[:], func=AF.Abs_reciprocal_sqrt
                    )
                    # alpha = cwu / sum_t cwu  (doubled except last iter)
                    cwsum = small_pool.tile([128, NCAPS], f32, tag="cwsum")
                    nc.vector.tensor_reduce(
                        out=cwsum[:], in_=cwu[:].transpose([0, 2, 1]), axis=X, op=ADD
                    )
                    rs = small_pool.tile([128, NCAPS], f32, tag="rs")
                    nc.vector.reciprocal(rs[:], cwsum[:])
                    if not last:
                        nc.vector.tensor_scalar_mul(rs[:], rs[:], 2.0)
                    alpha = small_pool.tile([128, KK, NCAPS], bf16, tag="alpha")
                    nc.vector.tensor_mul(
                        alpha[:],
                        cwu[:],
                        rs[:].unsqueeze(1).broadcast_to((128, KK, NCAPS)),
                    )

                    # o' = sum_t alpha_t p_t : product + tap halving
                    wprod = big_pool.tile([128, KK, CCH, NCAPS], bf16, tag="wp")
                    ab = alpha[:].unsqueeze(2).broadcast_to((128, KK, CCH, NCAPS))
                    nc.vector.tensor_mul(wprod[:], priors[:], ab)
                    wp = wprod[:].rearrange("p t a b -> p t (a b)")
                    nc.gpsimd.dma_start(
                        out=wp[:, 0:4], in_=wp[:, 4:8], accum_op=ACC
                    )
                    nc.gpsimd.dma_start(
                        out=wp[:, 0:2], in_=wp[:, 2:4], accum_op=ACC
                    )
                    wh = half_pool.tile([128, 1, NC], bf16, tag="wh")
                    nc.vector.tensor_add(wh[:, 0], wp[:, 0], wp[:, 1])
                    if not last:
                        o2 = o_pool.tile([128, NC], bf16)
                        nc.vector.tensor_add(o2[:], wh[:, 0], wp[:, 8])
                    else:
                        # write final o in natural (cap, cch) order, fp32
                        onat = o_pool.tile([128, NC], f32, tag="onat")
                        nc.vector.tensor_add(
                            onat[:].rearrange("p (cap cch) -> p cch cap", cch=CCH),
                            wh[:, 0].rearrange("p (cch cap) -> p cch cap", cch=CCH),
                            wp[:, 8].rearrange("p (cch cap) -> p cch cap", cch=CCH),
                        )

                # ---- transpose to [nc, pos] and store ----
                ot = small_pool.tile([128, 4, 128], f32, tag="ostage")
                for blk in range(4):
                    tp = tpp.tile([128, 128], f32)
                    nc.tensor.transpose(
                        tp[:], onat[:, 128 * blk : 128 * (blk + 1)], ident[:]
                    )
                    nc.scalar.copy(out=ot[:, blk], in_=tp[:])
                nc.sync.dma_start(
                    out=out_d[:, 128 * ch : 128 * (ch + 1)].rearrange(
                        "(blk n) q -> n blk q", blk=4
                    ),
                    in_=ot[:],
                )
    nc.compile()
    return nc


def _get_nc(with_bias: bool):
    key = ("nc", with_bias)
    if key not in _cache:
        _cache[key] = _build(with_bias)
    return _cache[key]


def kernel(input, weight, bias, _trace=False):
    from concourse.bass_utils import run_bass_kernel_spmd

    input = np.ascontiguousarray(np.asarray(input, dtype=np.float32))
    w = np.ascontiguousarray(
        np.asarray(weight, dtype=np.float32).reshape(KK, C, NC)
    )
    b = np.ascontiguousarray(np.asarray(bias, dtype=np.float32).reshape(KK, NC))
    with_bias = bool(np.any(b))

    nc = _get_nc(with_bias)
    in_maps = [
        {"x": np.ascontiguousarray(input[i]), "w": w, "b": b} for i in range(B)
    ]
    res = run_bass_kernel_spmd(
        nc, in_maps, core_ids=list(range(B)), trace=_trace
    )
    _cache["last_result"] = res
    out = np.stack(
        [r["out"].reshape(NC, H, W) for r in res.results], axis=0
    )
    return out


# revision 28
# speedup vs baseline: 1.9071x; 1.9071x over previous
"""Trainium2 Bass kernel for nn_CapsuleLayer (capsule conv + 3-iter routing).

Reference (per batch image, C=128, H=W=32, K=3, pad=1):
  priors[h,w,t,nc] = sum_c x_pad[c,h+i,w+j] * W[t,c,nc] + b[t,nc]
  o = mean_t priors
  3x: d2 = sum_cch (o - p_t)^2 ; cw = rsqrt(d2 + 1e-4)
      cw = cw / sum_t cw ; o = sum_t cw_t p_t
  out[nc,h,w] = o

Sharding: data-parallel over batch; 8 cores, one image each; weight/bias
replicated; no collectives.

Implementation notes:
- bf16 on-chip (fp32 PSUM accumulation in matmuls); rel err ~8e-3 vs the
  fp32 reference, within the 2e-2 gate.
- priors layout [128pos, grp, tap9, cch16, cap32]: innermost step-1 cap
  runs keep every big DVE op in 2x_1P mode; routing processes GRP=2
  position-chunks per pass to amortize per-op overhead.
- ||o - p||^2 = ||p||^2 - <2o, p> + ||o||^2: per iteration only two
  full-size DVE passes (product o2*p, product p*alpha) plus halving-add
  reductions (2x mode; 1x tensor_reduce avoided for bulk work).
- mean_t priors via 9 extra accumulating matmuls on the idle PE.
- rsqrt = Abs_reciprocal_sqrt on ACT: lives in one table set together
  with square/copy/identity -> no ACT table switching.
- ||o||^2 = sum_t alpha_t <o, p_t> reuses the s-values (no extra pass).
"""

import numpy as np

C = 128
H = W = 32
B = 8
KK = 9
NCAPS = 32
CCH = 16
NC = NCAPS * CCH  # 512
NIT = 3
NPOS = H * W
CHUNK = 128
GRP = 2  # position-chunks per routing pass
NGRP = NPOS // (CHUNK * GRP)
PADW = 34

_cache = {}


def _build(with_bias: bool):
    import concourse.bass as bass
    import concourse.tile as tile
    from concourse import bacc, mybir
    from concourse.masks import make_identity

    f32 = mybir.dt.float32
    bf16 = mybir.dt.bfloat16
    X = mybir.AxisListType.X
    ADD = mybir.AluOpType.add
    MULT = mybir.AluOpType.mult
    AF = mybir.ActivationFunctionType

    nc = bacc.Bacc()
    x_d = nc.dram_tensor("x", [C, H, W], f32, kind="ExternalInput")
    w_d = nc.dram_tensor("w", [KK, C, NC], f32, kind="ExternalInput")
    b_d = nc.dram_tensor("b", [KK, NC], f32, kind="ExternalInput")
    out_d = nc.dram_tensor("out", [NC, NPOS], f32, kind="ExternalOutput")

    with tile.TileContext(nc) as tc:
        with (
            tc.tile_pool(name="singles", bufs=1) as singles,
            tc.tile_pool(name="stage", bufs=1) as stage_pool,
            tc.tile_pool(name="priors", bufs=2) as priors_pool,
            tc.tile_pool(name="big", bufs=2) as big_pool,
            tc.tile_pool(name="half", bufs=2) as half_pool,
            tc.tile_pool(name="o", bufs=2) as o_pool,
            tc.tile_pool(name="small", bufs=3) as small_pool,
            tc.tile_pool(name="pp", bufs=4, space="PSUM") as pp,
            tc.tile_pool(name="mp", bufs=2, space="PSUM") as mp,
            tc.tile_pool(name="tpp", bufs=2, space="PSUM") as tpp,
        ):
            # ---- stage inputs: pad, cast to bf16, im2col, permute W ----
            xpadf = stage_pool.tile([C, PADW * PADW], f32, tag="fstage")
            nc.gpsimd.memset(xpadf, 0.0)
            xpadf_v = xpadf[:].rearrange("p (h w) -> p h w", h=PADW)
            nc.sync.dma_start(out=xpadf_v[:, 1 : H + 1, 1 : W + 1], in_=x_d[:])
            xpad = singles.tile([C, PADW * PADW], bf16)
            nc.scalar.copy(out=xpad[:], in_=xpadf[:])
            xpad_v = xpad[:].rearrange("p (h w) -> p h w", h=PADW)

            xcol = []
            for t in range(KK):
                i, j = divmod(t, 3)
                xc = singles.tile([C, NPOS], bf16, tag=f"xcol{t}")
                nc.sync.dma_start(
                    out=xc[:].rearrange("p (h w) -> p h w", h=H),
                    in_=xpad_v[:, i : i + H, j : j + W],
                )
                xcol.append(xc)

            wraw = stage_pool.tile([C, KK, NC], f32, tag="fstage2")
            nc.sync.dma_start(out=wraw[:], in_=w_d[:].transpose([1, 0, 2]))
            # wsb[c, t, cch, cap] (bf16) <- wraw[c, t, cap*16+cch]
            wsb = singles.tile([C, KK, CCH, NCAPS], bf16)
            nc.scalar.copy(
                out=wsb[:],
                in_=wraw[:].rearrange("p t (cap cch) -> p t cch cap", cch=CCH),
            )

            ident = singles.tile([128, 128], f32)
            make_identity(nc, ident[:])

            eps = singles.tile([128, 1], f32)
            nc.gpsimd.memset(eps, 1e-4)

            if with_bias:
                braw = singles.tile([1, KK, NC], f32)
                nc.sync.dma_start(out=braw[:], in_=b_d[:].unsqueeze(0))
                bsb = singles.tile([1, KK, CCH, NCAPS], bf16)
                nc.scalar.copy(
                    out=bsb[:],
                    in_=braw[:].rearrange("p t (cap cch) -> p t cch cap", cch=CCH),
                )
                ones = singles.tile([1, CHUNK], bf16)
                nc.gpsimd.memset(ones, 1.0)

            for g in range(NGRP):
                # ---- priors + mean via PE ----
                priors = priors_pool.tile([128, GRP, KK, CCH, NCAPS], bf16)
                o2 = o_pool.tile([128, GRP, NC], bf16)
                for cc in range(GRP):
                    ch = GRP * g + cc
                    om = mp.tile([128, NC], f32)  # sum_t priors (fp32 psum)
                    for t in range(KK):
                        ps = pp.tile([128, NC], f32)
                        lhsT = xcol[t][:, CHUNK * ch : CHUNK * (ch + 1)]
                        rhs = wsb[:, t].rearrange("p a b -> p (a b)")
                        if with_bias:
                            nc.tensor.matmul(
                                ps[:], lhsT, rhs, start=True, stop=False
                            )
                            brhs = bsb[:, t].rearrange("p a b -> p (a b)")
                            nc.tensor.matmul(
                                ps[:], ones[:], brhs, start=False, stop=True
                            )
                        else:
                            nc.tensor.matmul(ps[:], lhsT, rhs, start=True, stop=True)
                        nc.tensor.matmul(
                            om[:], lhsT, rhs, start=(t == 0), stop=(t == KK - 1)
                        )
                        if with_bias:
                            nc.tensor.matmul(
                                om[:], ones[:], brhs, start=False, stop=False,
                                skip_group_check=True,
                            )
                        nc.scalar.copy(
                            out=priors[:, cc, t],
                            in_=ps[:].rearrange("p (a b) -> p a b", a=CCH),
                        )
                    # o2 = 2*mean = (2/9) sum_t priors  (bf16)
                    nc.scalar.activation(
                        out=o2[:, cc], in_=om[:], func=AF.Copy, scale=2.0 / KK
                    )

                # ---- n[t,cap] = sum_cch p^2 (once per group) ----
                tprod = big_pool.tile([128, GRP, KK, CCH, NCAPS], bf16, tag="big")
                nc.scalar.activation(out=tprod[:], in_=priors[:], func=AF.Square)
                h1 = half_pool.tile([128, GRP, KK, 8, NCAPS], bf16, tag="h1")
                nc.vector.tensor_add(
                    h1[:], tprod[:, :, :, 0:8], tprod[:, :, :, 8:16]
                )
                nc.vector.tensor_add(
                    h1[:, :, :, 0:4], h1[:, :, :, 0:4], h1[:, :, :, 4:8]
                )
                nc.vector.tensor_add(
                    h1[:, :, :, 0:2], h1[:, :, :, 0:2], h1[:, :, :, 2:4]
                )
                ntile = small_pool.tile([128, GRP, KK, NCAPS], bf16, tag="n")
                nc.vector.tensor_add(ntile[:], h1[:, :, :, 0], h1[:, :, :, 1])

                alpha = None
                for it in range(NIT):
                    last = it == NIT - 1
                    # s = <o2, p_t>: product + cch halving reduction
                    tprod = big_pool.tile(
                        [128, GRP, KK, CCH, NCAPS], bf16, tag="big"
                    )
                    ob = (
                        o2[:]
                        .rearrange("p c (a b) -> p c a b", a=CCH)
                        .unsqueeze(2)
                        .broadcast_to((128, GRP, KK, CCH, NCAPS))
                    )
                    nc.vector.tensor_mul(tprod[:], priors[:], ob)
                    h1 = half_pool.tile([128, GRP, KK, 8, NCAPS], bf16, tag="h1")
                    nc.vector.tensor_add(
                        h1[:], tprod[:, :, :, 0:8], tprod[:, :, :, 8:16]
                    )
                    nc.vector.tensor_add(
                        h1[:, :, :, 0:4], h1[:, :, :, 0:4], h1[:, :, :, 4:8]
                    )
                    nc.vector.tensor_add(
                        h1[:, :, :, 0:2], h1[:, :, :, 0:2], h1[:, :, :, 2:4]
                    )
                    s = small_pool.tile([128, GRP, KK, NCAPS], bf16, tag="s")
                    nc.vector.tensor_add(s[:], h1[:, :, :, 0], h1[:, :, :, 1])

                    # e2 = sum_t alpha_t s_t -> e' = scale*e2 + eps (ACT)
                    e2 = small_pool.tile([128, GRP, NCAPS], f32, tag="e2")
                    if alpha is None:
                        nc.vector.tensor_reduce(
                            out=e2[:],
                            in_=s[:].transpose([0, 1, 3, 2]),
                            axis=X,
                            op=ADD,
                        )
                        escale = 1.0 / (2 * KK)
                    else:
                        tm = small_pool.tile(
                            [128, GRP, KK, NCAPS], bf16, tag="tm"
                        )
                        nc.vector.tensor_mul(tm[:], alpha[:], s[:])
                        nc.vector.tensor_reduce(
                            out=e2[:],
                            in_=tm[:].transpose([0, 1, 3, 2]),
                            axis=X,
                            op=ADD,
                        )
                        escale = 0.25
                    e2b = small_pool.tile([128, GRP, NCAPS], bf16, tag="e2b")
                    nc.scalar.activation(
                        out=e2b[:], in_=e2[:], func=AF.Identity,
                        bias=eps[:], scale=escale,
                    )

                    # dist = (n - s) + e'  (bf16, 2x)
                    dist = small_pool.tile([128, GRP, KK, NCAPS], bf16, tag="dist")
                    nc.vector.tensor_sub(dist[:], ntile[:], s[:])
                    nc.vector.tensor_add(
                        dist[:],
                        dist[:],
                        e2b[:].unsqueeze(2).broadcast_to((128, GRP, KK, NCAPS)),
                    )
                    # cwu = dist^-0.5 (single-table-set rsqrt on ACT)
                    cwu = small_pool.tile([128, GRP, KK, NCAPS], bf16, tag="cwu")
                    nc.scalar.activation(
                        out=cwu[:], in_=dist[:], func=AF.Abs_reciprocal_sqrt
                    )
                    # alpha = cwu / sum_t cwu (doubled except last iter)
                    cwsum = small_pool.tile([128, GRP, NCAPS], f32, tag="cwsum")
                    nc.vector.tensor_reduce(
                        out=cwsum[:],
                        in_=cwu[:].transpose([0, 1, 3, 2]),
                        axis=X,
                        op=ADD,
                    )
                    rs = small_pool.tile([128, GRP, NCAPS], f32, tag="rs")
                    nc.vector.reciprocal(rs[:], cwsum[:])
                    rsb = small_pool.tile([128, GRP, NCAPS], bf16, tag="rsb")
                    nc.vector.tensor_scalar_mul(
                        rsb[:], rs[:], 1.0 if last else 2.0
                    )
                    alpha = small_pool.tile([128, GRP, KK, NCAPS], bf16, tag="al")
                    nc.vector.tensor_mul(
                        alpha[:],
                        cwu[:],
                        rsb[:].unsqueeze(2).broadcast_to((128, GRP, KK, NCAPS)),
                    )

                    # o' = sum_t alpha_t p_t: product + tap halving
                    wprod = big_pool.tile(
                        [128, GRP, KK, CCH, NCAPS], bf16, tag="big"
                    )
                    ab = alpha[:].unsqueeze(3).broadcast_to(
                        (128, GRP, KK, CCH, NCAPS)
                    )
                    nc.vector.tensor_mul(wprod[:], priors[:], ab)
                    wp = wprod[:].rearrange("p c t a b -> p c t (a b)")
                    wh = half_pool.tile([128, GRP, 4, NC], bf16, tag="wh")
                    nc.vector.tensor_add(wh[:], wp[:, :, 0:4], wp[:, :, 4:8])
                    nc.vector.tensor_add(
                        wh[:, :, 0:2], wh[:, :, 0:2], wh[:, :, 2:4]
                    )
                    nc.vector.tensor_add(wh[:, :, 0], wh[:, :, 0], wh[:, :, 1])
                    if not last:
                        o2 = o_pool.tile([128, GRP, NC], bf16)
                        nc.vector.tensor_add(o2[:], wh[:, :, 0], wp[:, :, 8])
                    else:
                        # final o in natural (cap, cch) order, fp32
                        onat = o_pool.tile([128, GRP, NC], f32, tag="onat")
                        nc.vector.tensor_add(
                            onat[:].rearrange(
                                "p c (cap cch) -> p c cch cap", cch=CCH
                            ),
                            wh[:, :, 0].rearrange(
                                "p c (cch cap) -> p c cch cap", cch=CCH
                            ),
                            wp[:, :, 8].rearrange(
                                "p c (cch cap) -> p c cch cap", cch=CCH
                            ),
                        )

                # ---- transpose to [nc, pos] and store ----
                for cc in range(GRP):
                    ch = GRP * g + cc
                    ot = small_pool.tile([128, 4, 128], f32, tag="ostage")
                    for blk in range(4):
                        tp = tpp.tile([128, 128], f32)
                        nc.tensor.transpose(
                            tp[:],
                            onat[:, cc, 128 * blk : 128 * (blk + 1)],
                            ident[:],
                        )
                        nc.scalar.copy(out=ot[:, blk], in_=tp[:])
                    nc.sync.dma_start(
                        out=out_d[:, 128 * ch : 128 * (ch + 1)].rearrange(
                            "(blk n) q -> n blk q", blk=4
                        ),
                        in_=ot[:],
                    )
    nc.compile()
    return nc


def _get_nc(with_bias: bool):
    key = ("nc", with_bias)
    if key not in _cache:
        _cache[key] = _build(with_bias)
    return _cache[key]


def kernel(input, weight, bias, _trace=False):
    from concourse.bass_utils import run_bass_kernel_spmd

    input = np.ascontiguousarray(np.asarray(input, dtype=np.float32))
    w = np.ascontiguousarray(
        np.asarray(weight, dtype=np.float32).reshape(KK, C, NC)
    )
    b = np.ascontiguousarray(np.asarray(bias, dtype=np.float32).reshape(KK, NC))
    with_bias = bool(np.any(b))

    nc = _get_nc(with_bias)
    in_maps = [
        {"x": np.ascontiguousarray(input[i]), "w": w, "b": b} for i in range(B)
    ]
    res = run_bass_kernel_spmd(
        nc, in_maps, core_ids=list(range(B)), trace=_trace
    )
    _cache["last_result"] = res
    out = np.stack(
        [r["out"].reshape(NC, H, W) for r in res.results], axis=0
    )
    return out
